# revision 1
# baseline (speedup 1.0000x reference)
import os
import sys

sys.path.insert(0, "/opt/trn_rl_repo")
os.environ.setdefault("MYCRO_LOCAL_CACHE", "1")

import numpy as np
import ml_dtypes
from contextlib import ExitStack

BFNP = ml_dtypes.bfloat16

from concourse import bacc, bass, tile
from concourse.bass_utils import run_bass_kernel_spmd

mybir = bass.mybir
dt = mybir.dt
ts = bass.ts
AF = mybir.ActivationFunctionType
ALU = mybir.AluOpType

B, S, HID = 4, 4096, 2048
HD, NH, NKV, NF = 64, 32, 8, 128
HDP = HD + 2               # even lane width (fp32r matmul needs even free dims)
EPS = 1e-4
NHC, NKVC = 16, 4          # per-core q heads / kv heads
TB = 512                   # tokens per block
NK = HID // 128            # 16 hid chunks
F32 = dt.float32
R = dt.float32r
BF = dt.bfloat16


def build_nc(s=S):
    nb = s // TB
    nc = bacc.Bacc()
    nc._allow_low_precision_reason = "bf16 matmul inputs; fp32 psum accumulation"
    xt = nc.declare_dram_parameter("xt", [HID, s], BF, False)
    wqt = nc.declare_dram_parameter("wqt", [HID, NHC * HD], BF, False)
    wkt = nc.declare_dram_parameter("wkt", [HID, NKVC * HD], BF, False)
    wvt = nc.declare_dram_parameter("wvt", [HID, NKVC * HD], BF, False)
    wot = nc.declare_dram_parameter("wot", [NHC * HD, HID], BF, False)
    cosr = nc.declare_dram_parameter("cosr", [128, s], F32, False)
    sinr = nc.declare_dram_parameter("sinr", [128, s], F32, False)
    pjt2 = nc.declare_dram_parameter("pjt2", [128, NF], BF, False)
    pjs2 = nc.declare_dram_parameter("pjs2", [128, NF], BF, False)
    e64 = nc.declare_dram_parameter("e64", [1, NKVC * HDP], R, False)
    ones_d = nc.declare_dram_parameter("ones_d", [128, TB], R, False)
    onec = nc.declare_dram_parameter("onec", [128, NKVC * 2], BF, False)
    out_d = nc.declare_dram_parameter("out", [s, HID], BF, True)

    with tile.TileContext(nc) as tc, ExitStack() as ctx:
        qpd = ctx.enter_context(tc.tile_pool(name="qpd", bufs=1, space="DRAM"))
        qpst = qpd.tile([NHC * NF, s], BF)
        pers = ctx.enter_context(tc.tile_pool(name="pers", bufs=1))
        pjt_sb = pers.tile([128, NF], BF)
        pjs_sb = pers.tile([128, NF], BF)
        kv_sb = pers.tile([NF, NKVC, HDP], R)
        kvb = pers.tile([NF, NKVC, HDP], BF)
        ecat = pers.tile([2, NKVC, HDP], R)
        ones_sb = pers.tile([128, TB], R)
        nc.sync.dma_start(pjt_sb[:], pjt2[:])
        nc.sync.dma_start(pjs_sb[:], pjs2[:])
        nc.sync.dma_start(ecat[1:2, :], e64[:])
        nc.sync.dma_start(ones_sb[:], ones_d[:])

        with tc.tile_pool(name="cosin", bufs=1) as csp:
            cos_sb = csp.tile([128, s], F32)
            sin_sb = csp.tile([128, s], F32)
            nc.sync.dma_start(cos_sb[:], cosr[:])
            nc.sync.dma_start(sin_sb[:], sinr[:])

            # ---------------- Pass A1: k, v, kv state ----------------
            with tc.tile_pool(name="wkv", bufs=1) as wkvp, \
                 tc.tile_pool(name="xa1", bufs=2) as xp, \
                 tc.tile_pool(name="a1w", bufs=2) as wp, \
                 tc.tile_pool(name="pk", bufs=2, space="PSUM") as pkp, \
                 tc.tile_pool(name="pkv", bufs=1, space="PSUM") as pkvp:
                wk_sb = wkvp.tile([128, NK, NKVC * HD], BF)
                wv_sb = wkvp.tile([128, NK, NKVC * HD], BF)
                for k in range(NK):
                    nc.sync.dma_start(wk_sb[:, k], wkt[ts(k, 128), :])
                    nc.sync.dma_start(wv_sb[:, k], wvt[ts(k, 128), :])
                kvps = pkvp.tile([NF, NKVC, HDP], F32)
                for t in range(nb):
                    x_sb = xp.tile([128, NK, TB], BF, tag="x")
                    for k in range(NK):
                        nc.sync.dma_start(x_sb[:, k], xt[ts(k, 128), ts(t, TB)])
                    # v token-major: [128 tok, 4 heads, 64] + ones column
                    v_sb = []
                    for c in range(4):
                        vp = pkp.tile([128, NKVC, HD], F32, tag="vp")
                        for k in range(NK):
                            nc.tensor.matmul(vp[:], lhsT=x_sb[:, k, ts(c, 128)],
                                             rhs=wv_sb[:, k],
                                             start=(k == 0), stop=(k == NK - 1))
                        vb = wp.tile([128, NKVC, HDP], BF, tag="vsb", bufs=5)
                        nc.sync.dma_start(vb[:, :, HD:HDP], onec[:])
                        nc.vector.tensor_copy(vb[:, :, 0:HD], vp[:])
                        v_sb.append(vb)
                    # k feature-major, 2 M-tiles of 2 heads each
                    for m in range(2):
                        kfp = pkp.tile([128, TB], F32, tag="kf")
                        for k in range(NK):
                            nc.tensor.matmul(kfp[:], lhsT=wk_sb[:, k, ts(m, 128)],
                                             rhs=x_sb[:, k],
                                             start=(k == 0), stop=(k == NK - 1))
                        p1 = wp.tile([128, TB], BF, tag="p1k")
                        p2 = wp.tile([128, TB], BF, tag="p2k")
                        nc.vector.tensor_mul(p1[:], kfp[:], cos_sb[:, ts(t, TB)])
                        nc.vector.tensor_mul(p2[:], kfp[:], sin_sb[:, ts(t, TB)])
                        for hh in range(2):
                            h = 2 * m + hh
                            o = 64 * hh
                            kpp = pkp.tile([128, 4, NF], F32, tag="kp")
                            for c in range(4):
                                nc.tensor.matmul(kpp[:, c], lhsT=p1[o:o + 64, ts(c, 128)],
                                                 rhs=pjt_sb[o:o + 64, :],
                                                 start=True, stop=False)
                                nc.tensor.matmul(kpp[:, c], lhsT=p2[o:o + 64, ts(c, 128)],
                                                 rhs=pjs_sb[o:o + 64, :],
                                                 start=False, stop=True)
                            kps = wp.tile([128, 4, NF], BF, tag="kps")
                            nc.scalar.activation(kps[:], kpp[:], AF.Relu)
                            for c in range(4):
                                # start only on the very first matmul into this
                                # psum bank: the start bit marks the whole 2KB
                                # zero region pending-zero, so a per-head start
                                # would wipe sibling heads' partial sums.
                                nc.tensor.matmul(kvps[:, h], lhsT=kps[:, c],
                                                 rhs=v_sb[c][:, h],
                                                 start=(t == 0 and h == 0 and c == 0),
                                                 stop=(t == nb - 1 and c == 3),
                                                 skip_group_check=True)
                nc.scalar.copy(kv_sb[:], kvps[:])
                nc.scalar.copy(kvb[:], kvps[:])
                # ecat row 0 = EPS * colsum_f(kv)   (row 1 holds 1e-6 at d=64)
                csum = pkvp.tile([1, NKVC, HDP], F32, tag="cs")
                nc.tensor.matmul(csum[:], lhsT=ones_sb[:, 0:1], rhs=kv_sb[:],
                                 start=True, stop=True)
                nc.scalar.activation(ecat[0:1, :], csum[:], AF.Copy, scale=EPS)

            # ---------------- Pass A2: q -> relu(phi) stash ----------------
            with tc.tile_pool(name="wq", bufs=1) as wqp, \
                 tc.tile_pool(name="xa2", bufs=2) as xp2, \
                 tc.tile_pool(name="a2w", bufs=2) as wp2, \
                 tc.tile_pool(name="pq", bufs=2, space="PSUM") as pqp, \
                 tc.tile_pool(name="pqp", bufs=2, space="PSUM") as pqpp:
                wq_sb = wqp.tile([128, NK, NHC * HD], BF)
                for k in range(NK):
                    nc.sync.dma_start(wq_sb[:, k], wqt[ts(k, 128), :])
                for t in range(nb):
                    x_sb = xp2.tile([128, NK, TB], BF, tag="x")
                    for k in range(NK):
                        nc.sync.dma_start(x_sb[:, k], xt[ts(k, 128), ts(t, TB)])
                    for qt in range(8):
                        qf = pqp.tile([128, TB], F32, tag="qf")
                        for k in range(NK):
                            nc.tensor.matmul(qf[:], lhsT=wq_sb[:, k, ts(qt, 128)],
                                             rhs=x_sb[:, k],
                                             start=(k == 0), stop=(k == NK - 1))
                        p1 = wp2.tile([128, TB], BF, tag="p1q")
                        p2 = wp2.tile([128, TB], BF, tag="p2q")
                        nc.vector.tensor_mul(p1[:], qf[:], cos_sb[:, ts(t, TB)])
                        nc.vector.tensor_mul(p2[:], qf[:], sin_sb[:, ts(t, TB)])
                        for h2 in range(2):
                            o = 64 * h2
                            qpp = pqpp.tile([NF, TB], F32, tag="qp")
                            nc.tensor.matmul(qpp[:], lhsT=pjt_sb[o:o + 64, :],
                                             rhs=p1[o:o + 64, :],
                                             start=True, stop=False)
                            nc.tensor.matmul(qpp[:], lhsT=pjs_sb[o:o + 64, :],
                                             rhs=p2[o:o + 64, :],
                                             start=False, stop=True)
                            qsb = wp2.tile([NF, TB], BF, tag="qsb", bufs=3)
                            nc.scalar.activation(qsb[:], qpp[:], AF.Relu)
                            nc.sync.dma_start(qpst[ts(2 * qt + h2, NF), ts(t, TB)], qsb[:])

        # ---------------- Pass B: num/den, divide, o_proj ----------------
        with tc.tile_pool(name="wo", bufs=1) as wop, \
             tc.tile_pool(name="qpin", bufs=2) as qip, \
             tc.tile_pool(name="bw", bufs=2) as wp3, \
             tc.tile_pool(name="pn", bufs=3, space="PSUM") as pnp, \
             tc.tile_pool(name="pr", bufs=2, space="PSUM") as prp, \
             tc.tile_pool(name="po", bufs=2, space="PSUM") as pop:
            wo_sb = wop.tile([128, 8, HID], BF)
            for p in range(8):
                nc.sync.dma_start(wo_sb[:, p], wot[ts(p, 128), :])
            for t in range(nb):
                qp_sb = qip.tile([NF, NHC, TB], BF, tag="qpb")
                for h in range(NHC):
                    nc.sync.dma_start(qp_sb[:, h], qpst[ts(h, NF), ts(t, TB)])
                attn = wp3.tile([128, 8, TB], BF, tag="attn")
                for h in range(NHC):
                    j = h // 4
                    nh = pnp.tile([HDP, TB], F32, tag="nps")
                    nc.tensor.matmul(nh[:], lhsT=ecat[:, j], rhs=ones_sb[0:2, :],
                                     start=True, stop=False)
                    nc.tensor.matmul(nh[:], lhsT=kvb[:, j], rhs=qp_sb[:, h],
                                     start=False, stop=True)
                    dsb = wp3.tile([1, TB], F32, tag="dsb", bufs=3)
                    nc.scalar.copy(dsb[:], nh[HD:HD + 1, :])
                    rsb = wp3.tile([1, TB], R, tag="rsb", bufs=3)
                    with nc.allow_low_precision(reason="f32r bits == f32 bits"):
                        nc.vector.reciprocal(rsb[:], dsb[:])
                    rb = prp.tile([HD, TB], F32, tag="rbc")
                    nc.tensor.matmul(rb[:], lhsT=ones_sb[0:1, 0:HD], rhs=rsb[:],
                                     start=True, stop=True)
                    rbs = wp3.tile([HD, TB], F32, tag="rbs", bufs=3)
                    nc.scalar.copy(rbs[:], rb[:])
                    nc.vector.tensor_mul(attn[64 * (h % 2):64 * (h % 2) + 64, h // 2, :],
                                         nh[0:HD, :], rbs[:])
                for ct in range(4):
                    for n in range(4):
                        ops = pop.tile([128, TB], F32, tag="op")
                        for p in range(8):
                            nc.tensor.matmul(ops[:], lhsT=attn[:, p, ts(ct, 128)],
                                             rhs=wo_sb[:, p, ts(n, TB)],
                                             start=(p == 0), stop=(p == 7))
                        osb = wp3.tile([128, TB], BF, tag="osb", bufs=3)
                        nc.scalar.copy(osb[:], ops[:])
                        nc.sync.dma_start(out_d[512 * t + 128 * ct:512 * t + 128 * (ct + 1),
                                                ts(n, TB)], osb[:])
    nc.finalize()
    return nc


def make_in_maps(cos, sin, hidden_states, w_qkv, w_o, proj):
    cos = np.ascontiguousarray(cos, np.float32)
    sin = np.ascontiguousarray(sin, np.float32)
    hidden_states = np.asarray(hidden_states, np.float32)
    w_qkv = np.asarray(w_qkv, np.float32)
    w_o = np.asarray(w_o, np.float32)
    proj = np.asarray(proj, np.float32)
    s = hidden_states.shape[1]
    scale = (1.0 / np.sqrt(NF)) * (1.0 / (np.sqrt(HD) + EPS))
    pjt = (scale * proj.T).astype(np.float32)            # [64, 128]
    pjs = np.roll(pjt, -32, axis=0)
    pjt2 = np.ascontiguousarray(np.tile(pjt, (2, 1))).astype(BFNP)  # [128, 128]
    pjs2 = np.ascontiguousarray(np.tile(pjs, (2, 1))).astype(BFNP)
    sinsig = np.empty_like(sin)
    sinsig[:, :32] = sin[:, 32:]
    sinsig[:, 32:] = -sin[:, :32]
    cosr = np.ascontiguousarray(np.tile(cos.T, (2, 1)))  # [128, s]
    sinr = np.ascontiguousarray(np.tile(sinsig.T, (2, 1)))
    e64 = np.zeros((1, NKVC * HDP), np.float32)
    for j in range(NKVC):
        e64[0, j * HDP + HD] = 1e-6
    ones_d = np.ones((128, TB), np.float32)
    onec = np.ones((128, NKVC * 2), BFNP)
    in_maps = []
    for b in range(hidden_states.shape[0]):
        xtb = np.ascontiguousarray(hidden_states[b].T).astype(BFNP)
        for g in range(2):
            in_maps.append({
                "xt": xtb,
                "wqt": np.ascontiguousarray(w_qkv[g * 1024:(g + 1) * 1024, :].T).astype(BFNP),
                "wkt": np.ascontiguousarray(w_qkv[2048 + g * 256:2048 + (g + 1) * 256, :].T).astype(BFNP),
                "wvt": np.ascontiguousarray(w_qkv[2560 + g * 256:2560 + (g + 1) * 256, :].T).astype(BFNP),
                "wot": np.ascontiguousarray(w_o[:, g * 1024:(g + 1) * 1024].T).astype(BFNP),
                "cosr": cosr, "sinr": sinr, "pjt2": pjt2, "pjs2": pjs2, "e64": e64,
                "ones_d": ones_d, "onec": onec,
            })
    return in_maps


def run(inputs, trace=False):
    in_maps = make_in_maps(**inputs)
    s = in_maps[0]["xt"].shape[1]
    nc = build_nc(s)
    res = run_bass_kernel_spmd(nc, in_maps, list(range(8)), trace=trace)
    outs = [np.asarray(r["out"]).astype(np.float32) for r in res.results]
    full = np.stack([outs[2 * b] + outs[2 * b + 1] for b in range(len(outs) // 2)], 0)
    return full.astype(np.float32), res


def kernel(**inputs):
    out, _ = run(inputs, trace=False)
    return out



# revision 14
# speedup vs baseline: 55.2841x; 55.2841x over previous
import os
import sys

sys.path.insert(0, "/opt/trn_rl_repo")
os.environ.setdefault("MYCRO_LOCAL_CACHE", "1")

import numpy as np
import ml_dtypes
from contextlib import ExitStack

BFNP = ml_dtypes.bfloat16

from concourse import bacc, bass, tile
from concourse.bass_utils import run_bass_kernel_spmd

mybir = bass.mybir
dt = mybir.dt
ts = bass.ts
AF = mybir.ActivationFunctionType
ALU = mybir.AluOpType

B, S, HID = 4, 4096, 2048
HD, NH, NKV, NF = 64, 32, 8, 128
HDP = HD + 2               # even lane width (fp32r matmul needs even free dims)
EPS = 1e-4
NHC, NKVC = 16, 4          # per-core q heads / kv heads
TB = 512                   # tokens per block
NK = HID // 128            # 16 hid chunks
F32 = dt.float32
R = dt.float32r
BF = dt.bfloat16


def build_nc(s=S, loop_n=1):
    nb = s // TB
    nc = bacc.Bacc()
    nc._allow_low_precision_reason = "bf16 matmul inputs; fp32 psum accumulation"
    xt = nc.declare_dram_parameter("xt", [HID, s], BF, False)
    wqt = nc.declare_dram_parameter("wqt", [HID, NHC * HD], BF, False)
    wkt = nc.declare_dram_parameter("wkt", [HID, NKVC * HD], BF, False)
    wvt = nc.declare_dram_parameter("wvt", [HID, NKVC * HD], BF, False)
    wot = nc.declare_dram_parameter("wot", [NHC * HD, HID], BF, False)
    cosr = nc.declare_dram_parameter("cosr", [128, s], BF, False)
    sinr = nc.declare_dram_parameter("sinr", [128, s], BF, False)
    pjt2 = nc.declare_dram_parameter("pjt2", [128, NF], BF, False)
    pjs2 = nc.declare_dram_parameter("pjs2", [128, NF], BF, False)
    ecol = nc.declare_dram_parameter("ecol", [HDP, 1], F32, False)
    ones_d = nc.declare_dram_parameter("ones_d", [128, HD], R, False)
    onec = nc.declare_dram_parameter("onec", [128, NKVC * 2], BF, False)
    out_d = nc.declare_dram_parameter("out", [s, HID], BF, True)

    with tile.TileContext(nc) as tc, ExitStack() as ctx:
        if loop_n > 1:
            ctx.enter_context(tc.For_i(0, loop_n, 1))
        qpd = ctx.enter_context(tc.tile_pool(name="qpd", bufs=1, space="DRAM"))
        qpst = qpd.tile([NHC * NF, s], BF)
        pers = ctx.enter_context(tc.tile_pool(name="pers", bufs=1))
        pjt_sb = pers.tile([128, NF], BF)
        pjs_sb = pers.tile([128, NF], BF)
        kv_sb = pers.tile([NF, NKVC, HDP], R)
        kvb = pers.tile([NF, NKVC, HDP], BF)
        cv_sb = pers.tile([HDP, NKVC], F32)
        ecol_sb = pers.tile([HDP, 1], F32)
        ones_sb = pers.tile([128, HD], R)
        onec_sb = pers.tile([128, 2], BF)
        wo_sb = pers.tile([128, 8, HID], BF)

        # ---------------- Pass A: qkv proj, phi, kv state, qp stash --------
        with tc.tile_pool(name="wkv", bufs=1) as wkvp, \
             tc.tile_pool(name="csn", bufs=2) as csp, \
             tc.tile_pool(name="xa", bufs=2) as xp, \
             tc.tile_pool(name="aw", bufs=2) as wp, \
             tc.tile_pool(name="pproj", bufs=4, space="PSUM") as ppp, \
             tc.tile_pool(name="pphi", bufs=2, space="PSUM") as php, \
             tc.tile_pool(name="pkv", bufs=1, space="PSUM") as pkvp:
            wq_sb = wkvp.tile([128, NK, NHC * HD], BF)
            wk_sb = wkvp.tile([128, NK, NKVC * HD], BF)
            wv_sb = wkvp.tile([128, NK, NKVC * HD], BF)
            kvps = pkvp.tile([NF, NKVC, HDP], F32)
            for t in range(nb):
                x_sb = xp.tile([128, NK, TB], BF, tag="x")
                for c in range(4):
                    nc.sync.dma_start(x_sb[:, ts(c, 4)],
                                      xt[ts(c, 512), ts(t, TB)].rearrange(
                                          "(k p) t -> p k t", k=4))
                    if t == 0:
                        nc.scalar.dma_start(wv_sb[:, ts(c, 4)],
                                            wvt[ts(c, 512), :].rearrange(
                                                "(k p) t -> p k t", k=4))
                        nc.scalar.dma_start(wk_sb[:, ts(c, 4)],
                                            wkt[ts(c, 512), :].rearrange(
                                                "(k p) t -> p k t", k=4))
                        nc.scalar.dma_start(wq_sb[:, ts(c, 4)],
                                            wqt[ts(c, 512), :].rearrange(
                                                "(k p) t -> p k t", k=4))
                cos_sb = csp.tile([128, TB], BF, tag="cos")
                sin_sb = csp.tile([128, TB], BF, tag="sin")
                nc.scalar.dma_start(cos_sb[:], cosr[:, ts(t, TB)])
                nc.scalar.dma_start(sin_sb[:], sinr[:, ts(t, TB)])
                if t == 0:
                    nc.scalar.dma_start(pjt_sb[:], pjt2[:])
                    nc.scalar.dma_start(pjs_sb[:], pjs2[:])
                    nc.scalar.dma_start(ecol_sb[:], ecol[:])
                    nc.scalar.dma_start(ones_sb[:], ones_d[:])
                    nc.scalar.dma_start(onec_sb[:], onec[:, 0:2])
                if t == 1:
                    for c in range(2):
                        nc.scalar.dma_start(wo_sb[:, ts(c, 4)],
                                            wot[ts(c, 512), :].rearrange(
                                                "(k p) t -> p k t", k=4))
                # v token-major: [128 tok, 4 heads, 64] + ones column
                v_sb = []
                for c in range(4):
                    vp = ppp.tile([128, NKVC, HD], F32, tag="proj")
                    for k in range(NK):
                        nc.tensor.matmul(vp[:], lhsT=x_sb[:, k, ts(c, 128)],
                                         rhs=wv_sb[:, k],
                                         start=(k == 0), stop=(k == NK - 1))
                    vb = wp.tile([128, NKVC, HDP], BF, tag="vsb", bufs=5)
                    nc.gpsimd.dma_start(vb[:, :, HD:HDP], onec[:])
                    nc.vector.tensor_copy(vb[:, :, 0:HD], vp[:])
                    v_sb.append(vb)
                # k feature-major, 2 M-tiles of 2 heads each
                for m in range(2):
                    kfp = ppp.tile([128, TB], F32, tag="proj")
                    for k in range(NK):
                        nc.tensor.matmul(kfp[:], lhsT=wk_sb[:, k, ts(m, 128)],
                                         rhs=x_sb[:, k],
                                         start=(k == 0), stop=(k == NK - 1))
                    p1 = wp.tile([128, TB], BF, tag="p1k")
                    p2 = wp.tile([128, TB], BF, tag="p2k")
                    nc.vector.tensor_mul(p1[:], kfp[:], cos_sb[:])
                    nc.vector.tensor_mul(p2[:], kfp[:], sin_sb[:])
                    for hh in range(2):
                        h = 2 * m + hh
                        o = 64 * hh
                        kpp = php.tile([128, 4, NF], F32, tag="phi")
                        for c in range(4):
                            nc.tensor.matmul(kpp[:, c], lhsT=p1[o:o + 64, ts(c, 128)],
                                             rhs=pjt_sb[o:o + 64, :],
                                             start=True, stop=False)
                            nc.tensor.matmul(kpp[:, c], lhsT=p2[o:o + 64, ts(c, 128)],
                                             rhs=pjs_sb[o:o + 64, :],
                                             start=False, stop=True)
                        kps = wp.tile([128, 4, NF], BF, tag="kps")
                        nc.scalar.activation(kps[:], kpp[:], AF.Relu)
                        for c in range(4):
                            # start only on the very first matmul into this
                            # psum bank: the start bit marks the whole 2KB
                            # zero region pending-zero, so a per-head start
                            # would wipe sibling heads' partial sums.
                            nc.tensor.matmul(kvps[:, h], lhsT=kps[:, c],
                                             rhs=v_sb[c][:, h],
                                             start=(t == 0 and h == 0 and c == 0),
                                             stop=(t == nb - 1 and c == 3),
                                             skip_group_check=True)
                # q feature-major, 8 M-tiles of 2 heads each
                for qt in range(8):
                    qf = ppp.tile([128, TB], F32, tag="proj")
                    for k in range(NK):
                        nc.tensor.matmul(qf[:], lhsT=wq_sb[:, k, ts(qt, 128)],
                                         rhs=x_sb[:, k],
                                         start=(k == 0), stop=(k == NK - 1))
                    p1q = wp.tile([128, TB], BF, tag="p1q")
                    p2q = wp.tile([128, TB], BF, tag="p2q")
                    nc.vector.tensor_mul(p1q[:], qf[:], cos_sb[:])
                    nc.vector.tensor_mul(p2q[:], qf[:], sin_sb[:])
                    qsb = wp.tile([NF, 2, TB], BF, tag="qsb", bufs=3)
                    for h2 in range(2):
                        o = 64 * h2
                        qpp = php.tile([NF, TB], F32, tag="phi")
                        nc.tensor.matmul(qpp[:], lhsT=pjt_sb[o:o + 64, :],
                                         rhs=p1q[o:o + 64, :],
                                         start=True, stop=False)
                        nc.tensor.matmul(qpp[:], lhsT=pjs_sb[o:o + 64, :],
                                         rhs=p2q[o:o + 64, :],
                                         start=False, stop=True)
                        nc.scalar.activation(qsb[:, h2], qpp[:], AF.Relu)
                    nc.gpsimd.dma_start(qpst[ts(qt, 2 * NF), ts(t, TB)].rearrange(
                        "(h p) t -> p h t", h=2), qsb[:])
            # kv state copies + per-head bias vectors
            nc.scalar.copy(kv_sb[:], kvps[:])
            nc.scalar.copy(kvb[:], kvps[:])
            cvps = pkvp.tile([HDP, NKVC, 2], F32, tag="cv")
            for j in range(NKVC):
                nc.tensor.matmul(cvps[:, j], lhsT=kvb[:, j],
                                 rhs=onec_sb[:, 0:2],
                                 start=(j == 0), stop=(j == NKVC - 1),
                                 skip_group_check=True)
            # cv_sb[d, j] = EPS * sum_f kv[f, j, d] + (1e-6 at d==HD)
            nc.scalar.activation(cv_sb[:], cvps[:, :, 0], AF.Identity, scale=EPS,
                                 bias=ecol_sb[:, 0:1])

        # ---------------- Pass B: num/den, divide, o_proj ----------------
        with tc.tile_pool(name="qpin", bufs=2) as qip, \
             tc.tile_pool(name="bw", bufs=2) as wp3, \
             tc.tile_pool(name="pn", bufs=3, space="PSUM") as pnp, \
             tc.tile_pool(name="pr", bufs=2, space="PSUM") as prp, \
             tc.tile_pool(name="po", bufs=2, space="PSUM") as pop:
            for t in range(nb):
                qp_sb = qip.tile([NF, NHC, TB], BF, tag="qpb")
                for pr in range(8):
                    nc.sync.dma_start(qp_sb[:, ts(pr, 2)],
                                      qpst[ts(pr, 2 * NF), ts(t, TB)].rearrange(
                                          "(h p) t -> p h t", h=2))
                attn = wp3.tile([128, 8, TB], BF, tag="attn")
                for h in range(NHC):
                    j = h // 4
                    nh = pnp.tile([HDP, TB], F32, tag="nps")
                    nc.tensor.matmul(nh[:], lhsT=kvb[:, j], rhs=qp_sb[:, h],
                                     start=True, stop=True)
                    dsb = wp3.tile([1, TB], F32, tag="dsb", bufs=3)
                    nc.scalar.activation(dsb[:], nh[HD:HD + 1, :], AF.Identity,
                                         bias=cv_sb[HD:HD + 1, j:j + 1])
                    rsb = wp3.tile([1, TB], R, tag="rsb", bufs=3)
                    with nc.allow_low_precision(reason="f32r bits == f32 bits"):
                        nc.vector.reciprocal(rsb[:], dsb[:])
                    rb = prp.tile([HD, TB], F32, tag="rbc")
                    nc.tensor.matmul(rb[:], lhsT=ones_sb[0:1, :], rhs=rsb[:],
                                     start=True, stop=True)
                    nsb = wp3.tile([HD, TB], F32, tag="nsb", bufs=3)
                    nc.scalar.activation(nsb[:], nh[0:HD, :], AF.Identity,
                                         bias=cv_sb[0:HD, j:j + 1])
                    nc.vector.tensor_mul(attn[64 * (h % 2):64 * (h % 2) + 64, h // 2, :],
                                         nsb[:], rb[:])
                for ct in range(4):
                    osb = wp3.tile([128, 4, TB], BF, tag="osb", bufs=3)
                    for n in range(4):
                        ops = pop.tile([128, TB], F32, tag="op")
                        for p in range(8):
                            nc.tensor.matmul(ops[:], lhsT=attn[:, p, ts(ct, 128)],
                                             rhs=wo_sb[:, p, ts(n, TB)],
                                             start=(p == 0), stop=(p == 7))
                        nc.scalar.copy(osb[:, n], ops[:])
                    nc.gpsimd.dma_start(out_d[512 * t + 128 * ct:512 * t + 128 * (ct + 1), :],
                                        osb[:])
    nc.finalize()
    return nc


def make_in_maps(cos, sin, hidden_states, w_qkv, w_o, proj):
    cos = np.ascontiguousarray(cos, np.float32)
    sin = np.ascontiguousarray(sin, np.float32)
    hidden_states = np.asarray(hidden_states, np.float32)
    w_qkv = np.asarray(w_qkv, np.float32)
    w_o = np.asarray(w_o, np.float32)
    proj = np.asarray(proj, np.float32)
    s = hidden_states.shape[1]
    scale = (1.0 / np.sqrt(NF)) * (1.0 / (np.sqrt(HD) + EPS))
    pjt = (scale * proj.T).astype(np.float32)            # [64, 128]
    pjs = np.roll(pjt, -32, axis=0)
    pjt2 = np.ascontiguousarray(np.tile(pjt, (2, 1))).astype(BFNP)  # [128, 128]
    pjs2 = np.ascontiguousarray(np.tile(pjs, (2, 1))).astype(BFNP)
    sinsig = np.empty_like(sin)
    sinsig[:, :32] = sin[:, 32:]
    sinsig[:, 32:] = -sin[:, :32]
    cosr = np.ascontiguousarray(np.tile(cos.T, (2, 1))).astype(BFNP)  # [128, s]
    sinr = np.ascontiguousarray(np.tile(sinsig.T, (2, 1))).astype(BFNP)
    ecol = np.zeros((HDP, 1), np.float32)
    ecol[HD, 0] = 1e-6
    ones_d = np.ones((128, HD), np.float32)
    onec = np.ones((128, NKVC * 2), BFNP)
    in_maps = []
    for b in range(hidden_states.shape[0]):
        xtb = np.ascontiguousarray(hidden_states[b].T).astype(BFNP)
        for g in range(2):
            in_maps.append({
                "xt": xtb,
                "wqt": np.ascontiguousarray(w_qkv[g * 1024:(g + 1) * 1024, :].T).astype(BFNP),
                "wkt": np.ascontiguousarray(w_qkv[2048 + g * 256:2048 + (g + 1) * 256, :].T).astype(BFNP),
                "wvt": np.ascontiguousarray(w_qkv[2560 + g * 256:2560 + (g + 1) * 256, :].T).astype(BFNP),
                "wot": np.ascontiguousarray(w_o[:, g * 1024:(g + 1) * 1024].T).astype(BFNP),
                "cosr": cosr, "sinr": sinr, "pjt2": pjt2, "pjs2": pjs2,
                "ecol": ecol, "ones_d": ones_d, "onec": onec,
            })
    return in_maps


def run(inputs, trace=False):
    in_maps = make_in_maps(**inputs)
    s = in_maps[0]["xt"].shape[1]
    nc = build_nc(s)
    res = run_bass_kernel_spmd(nc, in_maps, list(range(8)), trace=trace)
    outs = [np.asarray(r["out"]).astype(np.float32) for r in res.results]
    full = np.stack([outs[2 * b] + outs[2 * b + 1] for b in range(len(outs) // 2)], 0)
    return full.astype(np.float32), res


def kernel(**inputs):
    out, _ = run(inputs, trace=False)
    return out


# revision 26
# speedup vs baseline: 56.7296x; 1.0261x over previous
import os
import sys

sys.path.insert(0, "/opt/trn_rl_repo")
os.environ.setdefault("MYCRO_LOCAL_CACHE", "1")

import numpy as np
import ml_dtypes
from contextlib import ExitStack

BFNP = ml_dtypes.bfloat16

from concourse import bacc, bass, tile
from concourse.bass_utils import run_bass_kernel_spmd

mybir = bass.mybir
dt = mybir.dt
ts = bass.ts
AF = mybir.ActivationFunctionType
ALU = mybir.AluOpType

B, S, HID = 4, 4096, 2048
HD, NH, NKV, NF = 64, 32, 8, 128
HDP = HD + 2               # even lane width (fp32r matmul needs even free dims)
EPS = 1e-4
NHC, NKVC = 16, 4          # per-core q heads / kv heads
TB = 512                   # tokens per block
NK = HID // 128            # 16 hid chunks
F32 = dt.float32
R = dt.float32r
BF = dt.bfloat16


def build_nc(s=S, loop_n=1):
    nb = s // TB
    nc = bacc.Bacc()
    nc._allow_low_precision_reason = "bf16 matmul inputs; fp32 psum accumulation"
    xt = nc.declare_dram_parameter("xt", [HID, s], BF, False)
    wqt = nc.declare_dram_parameter("wqt", [HID, NHC * HD], BF, False)
    wkt = nc.declare_dram_parameter("wkt", [HID, NKVC * HD], BF, False)
    wvt = nc.declare_dram_parameter("wvt", [HID, NKVC * HD], BF, False)
    wot = nc.declare_dram_parameter("wot", [NHC * HD, HID], BF, False)
    cosr = nc.declare_dram_parameter("cosr", [128, s], BF, False)
    sinr = nc.declare_dram_parameter("sinr", [128, s], BF, False)
    pjt2 = nc.declare_dram_parameter("pjt2", [128, NF], BF, False)
    pjs2 = nc.declare_dram_parameter("pjs2", [128, NF], BF, False)
    ecol = nc.declare_dram_parameter("ecol", [HDP, 1], F32, False)
    ones_d = nc.declare_dram_parameter("ones_d", [128, HD], R, False)
    onec = nc.declare_dram_parameter("onec", [128, NKVC * 2], BF, False)
    out_d = nc.declare_dram_parameter("out", [s, HID], BF, True)

    with tile.TileContext(nc) as tc, ExitStack() as ctx:
        if loop_n > 1:
            ctx.enter_context(tc.For_i(0, loop_n, 1))
        qpd = ctx.enter_context(tc.tile_pool(name="qpd", bufs=1, space="DRAM"))
        qpst = qpd.tile([NHC * NF, s], BF)
        pers = ctx.enter_context(tc.tile_pool(name="pers", bufs=1))
        pjt_sb = pers.tile([128, NF], BF)
        pjs_sb = pers.tile([128, NF], BF)
        kv_sb = pers.tile([NF, NKVC, HDP], R)
        kvb = pers.tile([NF, NKVC, HDP], BF)
        cv_sb = pers.tile([HDP, NKVC], F32)
        ecol_sb = pers.tile([HDP, 1], F32)
        ones_sb = pers.tile([128, HD], R)
        onec_sb = pers.tile([128, 2], BF)
        wo_sb = pers.tile([128, 8, HID], BF)
        cos_sb = pers.tile([128, s], BF)
        sin_sb = pers.tile([128, s], BF)
        qip = ctx.enter_context(tc.tile_pool(name="qpin", bufs=2))
        qp_pre = {}

        def load_qp(t):
            qp_sb = qip.tile([NF, NHC, TB], BF, tag="qpb")
            for pr in range(8):
                nc.sync.dma_start(qp_sb[:, ts(pr, 2)],
                                  qpst[ts(pr, 2 * NF), ts(t, TB)].rearrange(
                                      "(h p) t -> p h t", h=2))
            return qp_sb

        # ---------------- Pass A: qkv proj, phi, kv state, qp stash --------
        with tc.tile_pool(name="wkv", bufs=1) as wkvp, \
             tc.tile_pool(name="xa", bufs=2) as xp, \
             tc.tile_pool(name="aw", bufs=2) as wp, \
             tc.tile_pool(name="pproj", bufs=4, space="PSUM") as ppp, \
             tc.tile_pool(name="pphi", bufs=2, space="PSUM") as php, \
             tc.tile_pool(name="pkv", bufs=1, space="PSUM") as pkvp:
            wq_sb = wkvp.tile([128, NK, NHC * HD], BF)
            wk_sb = wkvp.tile([128, NK, NKVC * HD], BF)
            wv_sb = wkvp.tile([128, NK, NKVC * HD], BF)
            kvps = pkvp.tile([NF, NKVC, HDP], F32)
            for t in range(nb):
                x_sb = xp.tile([128, NK, TB], BF, tag="x")
                for c in range(4):
                    nc.sync.dma_start(x_sb[:, ts(c, 4)],
                                      xt[ts(c, 512), ts(t, TB)].rearrange(
                                          "(k p) t -> p k t", k=4))
                    if t == 0:
                        nc.scalar.dma_start(wv_sb[:, ts(c, 4)],
                                            wvt[ts(c, 512), :].rearrange(
                                                "(k p) t -> p k t", k=4))
                        nc.scalar.dma_start(wk_sb[:, ts(c, 4)],
                                            wkt[ts(c, 512), :].rearrange(
                                                "(k p) t -> p k t", k=4))
                        nc.scalar.dma_start(wq_sb[:, ts(c, 4)],
                                            wqt[ts(c, 512), :].rearrange(
                                                "(k p) t -> p k t", k=4))
                if t == 0:
                    nc.scalar.dma_start(cos_sb[:], cosr[:])
                    nc.scalar.dma_start(sin_sb[:], sinr[:])
                    nc.scalar.dma_start(pjt_sb[:], pjt2[:])
                    nc.scalar.dma_start(pjs_sb[:], pjs2[:])
                    nc.scalar.dma_start(ecol_sb[:], ecol[:])
                    nc.scalar.dma_start(ones_sb[:], ones_d[:])
                    nc.scalar.dma_start(onec_sb[:], onec[:, 0:2])
                if t == 1:
                    for c in range(2):
                        nc.scalar.dma_start(wo_sb[:, ts(c, 4)],
                                            wot[ts(c, 512), :].rearrange(
                                                "(k p) t -> p k t", k=4))
                # v token-major: [128 tok, 4 heads, 64] + ones column
                v_sb = []
                for c in range(4):
                    vp = ppp.tile([128, NKVC, HD], F32, tag="proj")
                    for k in range(NK):
                        nc.tensor.matmul(vp[:], lhsT=x_sb[:, k, ts(c, 128)],
                                         rhs=wv_sb[:, k],
                                         start=(k == 0), stop=(k == NK - 1))
                    vb = wp.tile([128, NKVC, HDP], BF, tag="vsb", bufs=5)
                    nc.gpsimd.memset(vb[:, :, HD:HDP], 1.0)
                    nc.vector.tensor_copy(vb[:, :, 0:HD], vp[:])
                    v_sb.append(vb)
                # k feature-major, 2 M-tiles of 2 heads each
                for m in range(2):
                    kfp = ppp.tile([128, TB], F32, tag="proj")
                    for k in range(NK):
                        nc.tensor.matmul(kfp[:], lhsT=wk_sb[:, k, ts(m, 128)],
                                         rhs=x_sb[:, k],
                                         start=(k == 0), stop=(k == NK - 1))
                    p1 = wp.tile([128, TB], BF, tag="p1k")
                    p2 = wp.tile([128, TB], BF, tag="p2k")
                    nc.vector.tensor_mul(p1[:], kfp[:], cos_sb[:, ts(t, TB)])
                    nc.vector.tensor_mul(p2[:], kfp[:], sin_sb[:, ts(t, TB)])
                    for hh in range(2):
                        h = 2 * m + hh
                        o = 64 * hh
                        kpp = php.tile([128, 4, NF], F32, tag="phi")
                        for c in range(4):
                            nc.tensor.matmul(kpp[:, c], lhsT=p1[o:o + 64, ts(c, 128)],
                                             rhs=pjt_sb[o:o + 64, :],
                                             start=True, stop=False)
                            nc.tensor.matmul(kpp[:, c], lhsT=p2[o:o + 64, ts(c, 128)],
                                             rhs=pjs_sb[o:o + 64, :],
                                             start=False, stop=True)
                        kps = wp.tile([128, 4, NF], BF, tag="kps")
                        nc.scalar.activation(kps[:], kpp[:], AF.Relu)
                        for c in range(4):
                            # start only on the very first matmul into this
                            # psum bank: the start bit marks the whole 2KB
                            # zero region pending-zero, so a per-head start
                            # would wipe sibling heads' partial sums.
                            nc.tensor.matmul(kvps[:, h], lhsT=kps[:, c],
                                             rhs=v_sb[c][:, h],
                                             start=(t == 0 and h == 0 and c == 0),
                                             stop=(t == nb - 1 and c == 3),
                                             skip_group_check=True)
                # q feature-major, 8 M-tiles of 2 heads each
                for qt in range(8):
                    qf = ppp.tile([128, TB], F32, tag="proj")
                    for k in range(NK):
                        nc.tensor.matmul(qf[:], lhsT=wq_sb[:, k, ts(qt, 128)],
                                         rhs=x_sb[:, k],
                                         start=(k == 0), stop=(k == NK - 1))
                    p1q = wp.tile([128, TB], BF, tag="p1q")
                    p2q = wp.tile([128, TB], BF, tag="p2q")
                    nc.vector.tensor_mul(p1q[:], qf[:], cos_sb[:, ts(t, TB)])
                    nc.vector.tensor_mul(p2q[:], qf[:], sin_sb[:, ts(t, TB)])
                    qsb = wp.tile([NF, 2, TB], BF, tag="qsb", bufs=3)
                    for h2 in range(2):
                        o = 64 * h2
                        qpp = php.tile([NF, TB], F32, tag="phi")
                        nc.tensor.matmul(qpp[:], lhsT=pjt_sb[o:o + 64, :],
                                         rhs=p1q[o:o + 64, :],
                                         start=True, stop=False)
                        nc.tensor.matmul(qpp[:], lhsT=pjs_sb[o:o + 64, :],
                                         rhs=p2q[o:o + 64, :],
                                         start=False, stop=True)
                        nc.scalar.activation(qsb[:, h2], qpp[:], AF.Relu)
                    nc.scalar.dma_start(qpst[ts(qt, 2 * NF), ts(t, TB)].rearrange(
                        "(h p) t -> p h t", h=2), qsb[:])
                if t == 0:
                    qp_pre[0] = load_qp(0)
            # kv state copies + per-head bias vectors
            nc.vector.tensor_copy(kv_sb[:], kvps[:])
            nc.vector.tensor_copy(kvb[:], kvps[:])
            cvps = pkvp.tile([HDP, NKVC, 2], F32, tag="cv")
            for j in range(NKVC):
                nc.tensor.matmul(cvps[:, j], lhsT=kvb[:, j],
                                 rhs=onec_sb[:, 0:2],
                                 start=(j == 0), stop=(j == NKVC - 1),
                                 skip_group_check=True)
            # cv_sb[d, j] = EPS * sum_f kv[f, j, d] + (1e-6 at d==HD)
            nc.scalar.activation(cv_sb[:], cvps[:, :, 0], AF.Identity, scale=EPS,
                                 bias=ecol_sb[:, 0:1])

        # ---------------- Pass B: num/den, divide, o_proj ----------------
        with tc.tile_pool(name="bw", bufs=2) as wp3, \
             tc.tile_pool(name="pn", bufs=3, space="PSUM") as pnp, \
             tc.tile_pool(name="pr", bufs=2, space="PSUM") as prp, \
             tc.tile_pool(name="po", bufs=2, space="PSUM") as pop:
            for t in range(nb):
                qp_sb = qp_pre.pop(t) if t in qp_pre else load_qp(t)
                attn = wp3.tile([128, 8, TB], BF, tag="attn")
                for h in range(NHC):
                    j = h // 4
                    nh = pnp.tile([HDP, TB], F32, tag="nps")
                    nc.tensor.matmul(nh[:], lhsT=kvb[:, j], rhs=qp_sb[:, h],
                                     start=True, stop=True)
                    dsb = wp3.tile([1, TB], F32, tag="dsb", bufs=3)
                    nc.scalar.activation(dsb[:], nh[HD:HD + 1, :], AF.Identity,
                                         bias=cv_sb[HD:HD + 1, j:j + 1])
                    rsb = wp3.tile([1, TB], R, tag="rsb", bufs=3)
                    with nc.allow_low_precision(reason="f32r bits == f32 bits"):
                        nc.vector.reciprocal(rsb[:], dsb[:])
                    rb = prp.tile([HD, TB], F32, tag="rbc")
                    nc.tensor.matmul(rb[:], lhsT=ones_sb[0:1, :], rhs=rsb[:],
                                     start=True, stop=True)
                    nsb = wp3.tile([HD, TB], F32, tag="nsb", bufs=3)
                    nc.scalar.activation(nsb[:], nh[0:HD, :], AF.Identity,
                                         bias=cv_sb[0:HD, j:j + 1])
                    nc.vector.tensor_mul(attn[64 * (h % 2):64 * (h % 2) + 64, h // 2, :],
                                         nsb[:], rb[:])
                for ct in range(4):
                    osb = wp3.tile([128, 4, TB], BF, tag="osb", bufs=3)
                    for n in range(4):
                        ops = pop.tile([128, TB], F32, tag="op")
                        for p in range(8):
                            nc.tensor.matmul(ops[:], lhsT=attn[:, p, ts(ct, 128)],
                                             rhs=wo_sb[:, p, ts(n, TB)],
                                             start=(p == 0), stop=(p == 7))
                        nc.scalar.copy(osb[:, n], ops[:])
                    nc.sync.dma_start(out_d[512 * t + 128 * ct:512 * t + 128 * (ct + 1), :],
                                      osb[:])
    nc.finalize()
    return nc


def make_in_maps(cos, sin, hidden_states, w_qkv, w_o, proj):
    cos = np.ascontiguousarray(cos, np.float32)
    sin = np.ascontiguousarray(sin, np.float32)
    hidden_states = np.asarray(hidden_states, np.float32)
    w_qkv = np.asarray(w_qkv, np.float32)
    w_o = np.asarray(w_o, np.float32)
    proj = np.asarray(proj, np.float32)
    s = hidden_states.shape[1]
    scale = (1.0 / np.sqrt(NF)) * (1.0 / (np.sqrt(HD) + EPS))
    pjt = (scale * proj.T).astype(np.float32)            # [64, 128]
    pjs = np.roll(pjt, -32, axis=0)
    pjt2 = np.ascontiguousarray(np.tile(pjt, (2, 1))).astype(BFNP)  # [128, 128]
    pjs2 = np.ascontiguousarray(np.tile(pjs, (2, 1))).astype(BFNP)
    sinsig = np.empty_like(sin)
    sinsig[:, :32] = sin[:, 32:]
    sinsig[:, 32:] = -sin[:, :32]
    cosr = np.ascontiguousarray(np.tile(cos.T, (2, 1))).astype(BFNP)  # [128, s]
    sinr = np.ascontiguousarray(np.tile(sinsig.T, (2, 1))).astype(BFNP)
    ecol = np.zeros((HDP, 1), np.float32)
    ecol[HD, 0] = 1e-6
    ones_d = np.ones((128, HD), np.float32)
    onec = np.ones((128, NKVC * 2), BFNP)
    in_maps = []
    for b in range(hidden_states.shape[0]):
        xtb = np.ascontiguousarray(hidden_states[b].T).astype(BFNP)
        for g in range(2):
            in_maps.append({
                "xt": xtb,
                "wqt": np.ascontiguousarray(w_qkv[g * 1024:(g + 1) * 1024, :].T).astype(BFNP),
                "wkt": np.ascontiguousarray(w_qkv[2048 + g * 256:2048 + (g + 1) * 256, :].T).astype(BFNP),
                "wvt": np.ascontiguousarray(w_qkv[2560 + g * 256:2560 + (g + 1) * 256, :].T).astype(BFNP),
                "wot": np.ascontiguousarray(w_o[:, g * 1024:(g + 1) * 1024].T).astype(BFNP),
                "cosr": cosr, "sinr": sinr, "pjt2": pjt2, "pjs2": pjs2,
                "ecol": ecol, "ones_d": ones_d, "onec": onec,
            })
    return in_maps


def run(inputs, trace=False):
    in_maps = make_in_maps(**inputs)
    s = in_maps[0]["xt"].shape[1]
    nc = build_nc(s)
    res = run_bass_kernel_spmd(nc, in_maps, list(range(8)), trace=trace)
    outs = [np.asarray(r["out"]).astype(np.float32) for r in res.results]
    full = np.stack([outs[2 * b] + outs[2 * b + 1] for b in range(len(outs) // 2)], 0)
    return full.astype(np.float32), res


def kernel(**inputs):
    out, _ = run(inputs, trace=False)
    return out
